# revision 40
# baseline (speedup 1.0000x reference)
"""v11 (4108ns, from 12267ns baseline): closed-form sparse rows +
prepared-SWDGE output writeback + sync/teardown surgery.

Algebra: with i=x_t[n], m=x_1[n], the relu'd numerator collapses —
for i==m the whole row is zero; for i!=m the only nonzero of the
pre-diag row is at j==m with value dk*s[i]/((1-k)*s[m]+k+eps), and
the diagonal correction puts its negation at j==i.  So per token the
device only builds row = val*(onehot(x1) - onehot(x_t)); the x_t==x_1
case needs no mask because onehot(x1)-onehot(x_t) is identically 0.

Packing: [128,256] — 2 partitions per token, halved vocab, shifted
indices (xt' = xt - 256*parity) compared against an on-device iota
(prefix scan of ones; gpsimd.iota hard-crashes the device).
s[x_t]*dk and s[x_1] are host-gathered into a tiny [128,8] input.

Output path: kv_writeback shaped as a dense [128,256]->[64,512] write.
Descriptors are PREPARED on the Pool engine at t~0 (gated only on the
int32 ctx-index memset), and the trigger fires as soon as the row
tile's final DVE op signals — skipping the HWDGE (625ns) and DGE->DMA
(650ns) stages a normal DMA pays after data-ready.

Sync surgery (post-tile, validated on device):
 - placeholder Pool waits retargeted to tile-assigned engine sems
 - kv completion routed to tile's DMASW accounting sem
 - same-engine DVE->DVE waits stripped (in-order queue + matched
   streaming rates make them redundant; first op keeps the DMA gate)
 - teardown: both exit all-engine barriers dropped, SP drain waits
   folded to the single load-bearing DMASW wait, Pool cleanup gated
   directly on DMASW
Framework trims: const-AP memsets rerouted from Pool to DVE; the
Bass.__init__ barrier is skipped (nothing reads the const APs).

Final schedule per core: input DMA latency 2.28us (fixed HWDGE+DGE+
sem-prop costs) | 0.78us DVE compute | trigger+transfer+sem 0.96us |
cleanup 0.07us.
"""
import numpy as np
from contextlib import ExitStack

N = 512
V = 512
NCORES = 8
NT = N // NCORES  # 64 tokens/core
P = 2 * NT        # 128 partitions
H = V // 2        # 256 free
EPS = 1e-8
W = 8             # packed input width


def build_default():
    import concourse.bass as bass
    import concourse.mybir as mybir
    from concourse import bacc
    from concourse import tile

    fp32 = mybir.dt.float32
    fp16 = mybir.dt.float16
    i32 = mybir.dt.int32
    Alu = mybir.AluOpType

    if not getattr(bass.BassGpSimd, "_v8_patched", False):
        def _memset_on_dve(self, ap, constant):
            return self.bass.vector.memset(ap, constant)

        bass.BassGpSimd.memset = _memset_on_dve
        bass.BassGpSimd._v8_patched = True

        _orig_barrier = bass.Bass.all_engine_barrier

        def _trimmed_barrier(self, *, sem_only=False):
            if not getattr(self, "_v8_saw_init_barrier", False):
                self._v8_saw_init_barrier = True
                return None
            return _orig_barrier(self, sem_only=True)

        bass.Bass.all_engine_barrier = _trimmed_barrier

    nc = bacc.Bacc("TRN2", target_bir_lowering=False, debug=False)

    pk_d = nc.dram_tensor("pk", [P, W], fp32, kind="ExternalInput")
    out_d = nc.dram_tensor("out", [NT, V], fp32, kind="ExternalOutput")

    # raw (non-pool) pieces of the writeback path: manual sems keep the
    # descriptor prep off the data-dependency chain.  The ctx-index memset
    # is emitted BEFORE the tile context: in the preamble block it runs
    # early, and outside tile's sem rewriting its manual inc survives (and
    # is its only sync update, which the engine-instruction limit allows).
    row_t = nc.alloc_sbuf_tensor("row_t", [P, H], fp32)
    ctxi = nc.alloc_sbuf_tensor("ctxi", [P, 1], i32)
    ctx_sem = nc.alloc_semaphore("ctx_sem")
    row_sem = nc.alloc_semaphore("row_sem")
    dma_sem = nc.alloc_semaphore("dma_sem")
    nc.vector.memset(ctxi.ap(), 0).then_inc(ctx_sem, 1)

    with tile.TileContext(nc) as tc, ExitStack() as ctx:
        pool = ctx.enter_context(tc.tile_pool(name="main", bufs=1))

        pk_t = pool.tile([P, W], fp32, name="pk_t")
        io_t = pool.tile([P, H], fp16, name="io_t")
        ones = pool.tile([P, H], fp16, name="ones")

        # prepared dense writeback: [1,128,1,256] view of out, ctx idx 0
        out_ap4 = (
            out_d.ap()
            .rearrange("a (h b) -> (a h) b", h=2)
            .rearrange("(x p) (y b) -> x p y b", x=1, y=1)
        )
        in_ap4 = row_t.ap().rearrange("p (x y b) -> p x y b", x=1, y=1)
        # tile's Pool_XX engine sem on the prep orders the trigger after it;
        # a manual prep_sem would be a third sync update (HW limit exceeded).
        # ctx wait is >=0 for tile's block-local deadlock sim (the inc lives
        # in the preamble block it can't see); bumped to >=1 post-tile.
        ctx_wait = nc.gpsimd.wait_ge(ctx_sem, 0)
        prep_inst = nc.gpsimd.kv_writeback(
            out_ap4, in_ap4, ctxi.ap(), prepare_only=True, sem=dma_sem
        )

        # iota via prefix scan of ones: no input deps, runs before the DMA
        nc.vector.memset(ones[:], 1.0)
        scan_inst = nc.vector.tensor_tensor_scan(
            io_t[:], ones[:], ones[:], -1.0, Alu.add, Alu.mult
        )
        nc.sync.dma_start(pk_t[:], pk_d.ap())

        xt_c, x1_c = pk_t[:, 0:1], pk_t[:, 1:2]
        omk_c, keps_c = pk_t[:, 2:3], pk_t[:, 3:4]
        sxtdk_c, sx1_c = pk_t[:, 4:5], pk_t[:, 5:6]

        dc = pool.tile([P, H], fp16, name="dc")
        di = pool.tile([P, H], fp16, name="di")
        d_t = pool.tile([P, H], fp16, name="d_t")

        def small(tag):
            return pool.tile([P, 1], fp32, name=tag)

        den, rec, val = small("den"), small("rec"), small("val")

        # small ops interleaved with the wide mask ops so the ~100ns
        # same-engine semaphore round-trips hide under engine-busy time
        nc.vector.tensor_scalar(den[:], sx1_c, omk_c, keps_c, Alu.mult, Alu.add)
        nc.vector.tensor_scalar(dc[:], io_t[:], x1_c, None, Alu.is_equal)
        nc.vector.reciprocal(rec[:], den[:])
        nc.vector.tensor_scalar(di[:], io_t[:], xt_c, None, Alu.is_equal)
        nc.vector.tensor_scalar(val[:], rec[:], sxtdk_c, None, Alu.mult)
        nc.vector.tensor_tensor(d_t[:], dc[:], di[:], Alu.subtract)

        row_inst = nc.vector.tensor_scalar(
            row_t.ap(), d_t[:], val[:], None, Alu.mult
        )

        # placeholder: trivially satisfiable for tile's exit-time deadlock
        # sim; rewritten below to the tile-assigned row-completion sem
        row_wait = nc.gpsimd.wait_ge(row_sem, 0)
        trig_inst = nc.gpsimd.trigger_dma(count=1)
        nc.sync.wait_ge(dma_sem, 16)

    # Tile's sem assignment erases manual then_inc on scheduled ops (and HW
    # allows only one sync update per engine instruction anyway), so waits on
    # op completion can't use private sems.  Instead: find the
    # engine-completion sem tile assigned to the op, count its cumulative
    # increments through the op, and retarget the placeholder wait.
    fn = nc.m.functions[0]

    def retarget(wait_handle, op_inst):
        upd = None
        for u in op_inst.ins.sync_info.on_update:
            if u.sync_type == "semaphore" and u.update_mode == "sem-inc":
                upd = u
                break
        assert upd is not None, f"no inc on {op_inst.ins.name}"
        total = 0
        for bb in fn.blocks:
            for inst in bb.instructions:
                si = getattr(inst, "sync_info", None)
                if si is not None:
                    for u in si.on_update:
                        if (
                            u.sync_type == "semaphore"
                            and u.id == upd.id
                            and u.update_mode == "sem-inc"
                        ):
                            total += u.update_value
                if inst.name == op_inst.ins.name:
                    return upd, total
        raise AssertionError(f"{op_inst.ins.name} not found")

    for wait_handle, op_inst in ((row_wait, row_inst),):
        upd, total = retarget(wait_handle, op_inst)
        w = wait_handle.ins.sync_info.on_wait[0]
        w.id = upd.id
        w.wait_value = total
        w.ant_name = upd.ant_name

    ctx_wait.ins.sync_info.on_wait[0].wait_value = 1

    # Strip same-engine DVE->DVE waits on the compute chain: the DVE queue is
    # in-order and producer/consumer stream at matching-or-slower rates, so
    # reads trail writes by a full op; the standalone DMA-wait EventSemaphore
    # earlier in the queue still gates the chain on the input DMA.  The row
    # op keeps its wait: it reads `val` as a scalar operand, fetched at op
    # setup rather than streamed.
    tile_bb = list(fn.blocks)[1]
    # Put the input-DMA wait directly on the first compute op (replacing its
    # program-sem wait) and delete the standalone DMA-wait EventSemaphore —
    # saves its decode from the data-ready -> compute handoff.
    dve_dma_ev = None
    for k, inst in enumerate(tile_bb.instructions):
        if (
            type(inst).__name__ == "InstEventSemaphore"
            and str(inst.engine).endswith("DVE")
            and inst.sync_info is not None
            and any(
                wt.ant_name and wt.ant_name.startswith("DMAHW")
                for wt in inst.sync_info.on_wait
            )
        ):
            dve_dma_ev = (k, inst)
            break
    first_kept = False
    for inst in tile_bb.instructions:
        if not str(inst.engine).endswith("DVE"):
            continue
        if type(inst).__name__ not in (
            "InstTensorScalarPtr", "InstTensorTensor", "InstReciprocal"
        ):
            continue
        if inst.name in (scan_inst.ins.name,):
            continue
        if not first_kept:
            first_kept = True
            if dve_dma_ev is not None:
                hw = next(
                    wt for wt in dve_dma_ev[1].sync_info.on_wait
                    if wt.ant_name and wt.ant_name.startswith("DMAHW")
                )
                fw = inst.sync_info.on_wait[0]
                fw.id = hw.id
                fw.ant_name = hw.ant_name
                fw.wait_mode = hw.wait_mode
                fw.wait_value = hw.wait_value
                del tile_bb.instructions[dve_dma_ev[0]]
            continue
        # row included: its last producer d immediately precedes it in queue
        # order (streamed tensor operand), and val completed an op earlier,
        # so even its setup-time scalar fetch is safe
        si = getattr(inst, "sync_info", None)
        if si is None:
            continue
        ws = si.on_wait
        while len(ws):
            del ws[0]

    # Tile's exit machinery expects the SWDGE queue-0 DMA to bump its own
    # DMASW0 sem; route the kv completion there and follow with our waits.
    dmasw_id, dmasw_name = None, None
    for bb in fn.blocks:
        for inst in bb.instructions:
            si = getattr(inst, "sync_info", None)
            if si is None:
                continue
            for wt in si.on_wait:
                if wt.ant_name and wt.ant_name.startswith("DMASW"):
                    dmasw_id, dmasw_name = wt.id, wt.ant_name
    assert dmasw_id is not None, "no DMASW exit wait found"
    pu = prep_inst.ins.sync_info.on_update[0]
    assert pu.ant_name == "dma_sem", pu
    pu.id, pu.ant_name = dmasw_id, dmasw_name
    for bb in fn.blocks:
        for inst in bb.instructions:
            si = getattr(inst, "sync_info", None)
            if si is None:
                continue
            for wt in si.on_wait:
                if wt.id == dma_sem.num:
                    wt.id, wt.ant_name = dmasw_id, dmasw_name
                # tile put its DVE-side DMA-drain wait BEFORE the row op in
                # the DVE stream -> cycle (DMA fires only after the row op).
                # SP's copy of the wait still gates the cleanup barrier, so
                # the drain guarantee survives neutralizing the others.
                if wt.id == dmasw_id and str(inst.engine) != "EngineType.SP":
                    wt.wait_value = 0

    # Teardown surgery on the exit block (bb.instructions is live):
    #  - fold the standalone SP drain-waits into the SP Drain
    #  - drop the post-cleanup second all-engine barrier: the Pool queue
    #    (ISA cleanup last) itself gates NEFF completion, so no engine can
    #    observe state from before the cleanup in a later invocation
    # Run both pre- and post-compile: bacc's finalize inserts the SP
    # drain-waits during compile, invisible to the pre-compile pass.
    def teardown_surgery():
        end_bb = list(fn.blocks)[-1]
        insts = end_bb.instructions
        sp_drain = next(
            (i for i in insts
             if type(i).__name__ == "InstDrain"
             and str(i.engine).endswith("SP")),
            None,
        )
        if sp_drain is None:
            return
        isa_pos = [
            k for k, i in enumerate(insts) if type(i).__name__ == "InstISA"
        ]
        pool_isa_pos = max(isa_pos) if isa_pos else len(insts)
        drain_pos = insts.index(sp_drain)
        # Gate the Pool cleanup directly on the kv DMA sem and drop the
        # pre-cleanup all-engine barrier: every other engine's queue has
        # already ended by then, and SP's Drain still guards its own queue.
        dmasw_wait = None
        for i in insts:
            si = getattr(i, "sync_info", None)
            if si is None:
                continue
            for wt in si.on_wait:
                if wt.ant_name and wt.ant_name.startswith("DMASW"):
                    dmasw_wait = wt
        pool_drain = next(
            (i for i in insts
             if type(i).__name__ == "InstDrain"
             and str(i.engine).endswith("Pool")),
            None,
        )
        if dmasw_wait is not None and pool_drain is not None:
            import concourse.mybir as _mybir

            nw = _mybir.SyncWait(
                sync_type="semaphore",
                id=dmasw_wait.id,
                ant_name=dmasw_wait.ant_name,
                wait_mode="sem-ge-imm",
                wait_value=16,
            )
            if pool_drain.sync_info is None:
                pool_drain.sync_info = _mybir.SyncInfo(
                    on_wait=[nw], on_update=[]
                )
            elif not pool_drain.sync_info.on_wait:
                pool_drain.sync_info.on_wait.append(nw)
        kill = []
        for k, i in enumerate(insts):
            if i.name.startswith("aeb_") and k < pool_isa_pos:
                kill.append(k)
                continue
            if (
                type(i).__name__ == "InstEventSemaphore"
                and str(i.engine).endswith("SP")
                and k < drain_pos
            ):
                # HW allows at most 2 waits per instruction.  Of the exit
                # waits only DMASW (the kv writeback) is load-bearing: it
                # fires after the row op, which already implies the input
                # DMA and all DVE work completed.
                # Drain carries exactly one wait slot: DMASW (kv DMA
                # completion) subsumes the Pool-tick wait tile put there
                # (the DMA can only fire after the prep ran).
                for wt in i.sync_info.on_wait:
                    if wt.ant_name and wt.ant_name.startswith("DMASW"):
                        dw = sp_drain.sync_info.on_wait[0]
                        dw.id = wt.id
                        dw.ant_name = wt.ant_name
                        dw.wait_mode = wt.wait_mode
                        dw.wait_value = wt.wait_value
                kill.append(k)
            elif k > pool_isa_pos and i.name.startswith("aeb_"):
                kill.append(k)
        for k in reversed(kill):
            del insts[k]

    teardown_surgery()
    nc.compile()
    teardown_surgery()
    return nc


def in_maps(source_p, k_t, d_k_t, x_t, x_1):
    s = np.asarray(source_p, dtype=np.float32).reshape(V)
    kf = np.float32(np.asarray(k_t).reshape(()))
    dkf = np.float32(np.asarray(d_k_t).reshape(()))
    xt = np.asarray(x_t).reshape(N).astype(np.int64)
    x1 = np.asarray(x_1).reshape(N).astype(np.int64)

    parity = np.tile(np.array([0, 1], dtype=np.int64), NT)  # per partition

    base = np.empty((P, W), dtype=np.float32)
    base[:, 2] = np.float32(1.0) - kf
    base[:, 3] = kf + np.float32(EPS)
    base[:, 6] = 0.0
    base[:, 7] = 0.0

    maps = []
    for c in range(NCORES):
        lo, hi = c * NT, (c + 1) * NT
        pk = base.copy()
        pk[:, 0] = (np.repeat(xt[lo:hi], 2) - H * parity).astype(np.float32)
        pk[:, 1] = (np.repeat(x1[lo:hi], 2) - H * parity).astype(np.float32)
        pk[:, 4] = np.repeat(s[xt[lo:hi]], 2) * dkf
        pk[:, 5] = np.repeat(s[x1[lo:hi]], 2)
        maps.append({"pk": pk})
    return maps


_CACHE = {}


def _get_nc():
    if "nc" not in _CACHE:
        _CACHE["nc"] = build_default()
    return _CACHE["nc"]


def _in_maps(source_p, k_t, d_k_t, x_t, x_1):
    return in_maps(source_p, k_t, d_k_t, x_t, x_1)


def kernel(source_p, k_t, d_k_t, x_t, x_1):
    from concourse.bass_utils import run_bass_kernel_spmd

    nc = _get_nc()
    maps = in_maps(source_p, k_t, d_k_t, x_t, x_1)
    res = run_bass_kernel_spmd(nc, maps, list(range(NCORES)))
    out = np.concatenate([res.results[c]["out"] for c in range(NCORES)], axis=0)
    return out.astype(np.float32)
